# revision 7
# baseline (speedup 1.0000x reference)
"""BiDense (binary dense) kernel for Trainium2, column-parallel over 8 NeuronCores.

Math (mirrors the reference exactly):
    bk[f] = max_d |kernel[d, f]| + f32_eps          (per-output-feature bound)
    bx[t] = max_d |x[t, d]|      + f32_eps          (per-token bound)
    kq = sign*(kernel) * 0.5 * bk[f]                (sign* maps 0 -> +1)
    xq = sign*(x)      * 0.5 * bx[t]
    y[t, f] = sum_d xq kq + bias[f]
            = 0.25 * bx[t] * bk[f] * (Sx @ Sk)[t, f] + bias[f]

Sx/Sk are +-1 matrices, so the GEMM runs exactly in fp8 (products are +-1,
accumulation of <=4096 integers is exact in fp32 PSUM).

Layout strategy (v2): the host pre-packs data layouts so the device-side
program is a pure fp8 DoubleRow GEMM pipeline with no PE transposes and no
weight-bound reduction chain:
  - x is repacked (pure layout permutation) to x^T tiles [j, p, kt, t] so the
    matmul lhsT (d on partitions) can be produced by a single ACT sign pass
    per token block - the PE never transposes.
  - kernel signs are packed to fp8 [p, kt, f] on the host (weight
    quantization), shrinking the weight stream 4x and making the first
    matmul runnable within microseconds of kernel start.
  - the tiny per-token / per-feature bounds vectors (0.02% of the FLOPs)
    are computed host-side and DMA'd as constants, so PSUM evacuation is
    never blocked on a bounds reduction.

Sharding: column-parallel (tensor-parallel over features).  Each core gets
the full x and a 1/8 slice of kernel/bias along f; outputs concat along f.
"""

import numpy as np
import ml_dtypes
from contextlib import ExitStack

import concourse.bass as bass
import concourse.mybir as mybir
import concourse.tile as tile
from concourse import bacc, bass_utils

P = 128
N_CORES = 8
F32_EPS = float(np.finfo(np.float32).eps)
SIGN_BIAS = 1e-30  # sign(v + tiny): maps v==0 to +1, never flips a real value

FP32 = mybir.dt.float32
FP8 = mybir.dt.float8e4
NP_FP8 = ml_dtypes.float8_e4m3
ALU = mybir.AluOpType


def build_nc(T, D, F, has_bias=False, NF=512, XB=4, SKQ=16, PRE=2):
    """Per-core Bass program: pure fp8 DoubleRow GEMM pipeline.

    T: tokens (full T), D: contraction, F: features on this core.
    Inputs (host-packed):
      xt_in  [T, D] fp32   row j*P+p, col kt*P+t  =  x[j*P+t, kt*P+p]
      sk_in  [P, KT, F] fp8  sign(k[kt*P+p, f])
      bx_in  [P, TB] fp32   bx[j*P+p] at [p, j]
      bkb_in [F] fp32       0.25*(bk[f]+eps)
    """
    KT, TB, FC = D // P, T // P, F // NF
    assert T % P == 0 and D % P == 0 and F % NF == 0 and KT % 2 == 0
    assert KT % SKQ == 0
    PRE = min(PRE, TB)

    nc = bacc.Bacc(trn_type="TRN2")
    xt_d = nc.dram_tensor("xt_in", [T, D], FP32, kind="ExternalInput")
    sk_d = nc.dram_tensor("sk_in", [P, KT, F], FP8, kind="ExternalInput")
    bx_d = nc.dram_tensor("bx_in", [P, TB], FP32, kind="ExternalInput")
    bkb_d = nc.dram_tensor("bkb_in", [F], FP32, kind="ExternalInput")
    b_d = None
    if has_bias:
        b_d = nc.dram_tensor("b_in", [F], FP32, kind="ExternalInput")
    y_d = nc.dram_tensor("y_out", [T, F], FP32, kind="ExternalOutput")

    with ExitStack() as ctx:
        tc = ctx.enter_context(tile.TileContext(nc))
        const = ctx.enter_context(tc.tile_pool(name="const", bufs=1))
        skp = ctx.enter_context(tc.tile_pool(name="sk", bufs=1))
        mmps = ctx.enter_context(tc.tile_pool(name="mmps", bufs=8, space="PSUM"))
        xp = ctx.enter_context(tc.tile_pool(name="xp", bufs=XB))
        sxtp = ctx.enter_context(tc.tile_pool(name="sxtp", bufs=XB))
        outp = ctx.enter_context(tc.tile_pool(name="outp", bufs=8))

        # weight/const stream on the gpsimd DMA ring so the x loads on the
        # sync ring are never queued behind it; chunked so early kt tiles
        # land first and block 0 can accumulate while the tail is in flight
        bxall = const.tile([P, TB], FP32)
        nc.gpsimd.dma_start(bxall, bx_d[:, :])
        bkb = const.tile([P, F], FP32)
        src = bkb_d[:]
        bcast = bass.AP(tensor=src.tensor, offset=src.offset,
                        ap=[[0, P]] + [list(pair) for pair in src.ap])
        nc.gpsimd.dma_start(bkb, bcast)
        biasb = None
        if has_bias:
            biasb = const.tile([P, F], FP32, name="biasb")
            bsrc = b_d[:]
            bb = bass.AP(tensor=bsrc.tensor, offset=bsrc.offset,
                         ap=[[0, P]] + [list(pair) for pair in bsrc.ap])
            nc.gpsimd.dma_start(biasb, bb)
        sk = skp.tile([P, KT, F], FP8)
        kq = KT // SKQ
        for q in range(SKQ):
            nc.gpsimd.dma_start(sk[:, q * kq:(q + 1) * kq, :],
                                sk_d[:, q * kq:(q + 1) * kq, :])
        sbias = const.tile([P, 1], FP32)   # tiny bias so sign(0+eps) = +1
        nc.vector.memset(sbias, SIGN_BIAS)

        pm = mybir.MatmulPerfMode.DoubleRow
        sxts = {}

        def emit_xload(j, chunks=1):
            # chunks>1 pipelines DMA->sign at kt-group granularity so block
            # 0's first matmul isn't gated on the whole 2MB tile + sign
            xt = xp.tile([P, D], FP32, tag="xt", name="xt")
            sxt = sxtp.tile([P, KT, P], FP8, tag="sxt", name="sxt")
            cw = D // chunks
            kw = KT // chunks
            for c in range(chunks):
                nc.sync.dma_start(xt[:, c * cw:(c + 1) * cw],
                                  xt_d[j * P:(j + 1) * P, c * cw:(c + 1) * cw])
                nc.scalar.sign(
                    sxt[:, c * kw:(c + 1) * kw, :],
                    xt[:, c * cw:(c + 1) * cw].rearrange(
                        "p (k t) -> p k t", k=kw),
                    bias=sbias[:])
            sxts[j] = sxt

        for j in range(PRE):
            emit_xload(j, chunks=(4 if j == 0 else 1))

        for j in range(TB):
            if j + PRE < TB:
                emit_xload(j + PRE)
            sxt = sxts.pop(j)
            mm = [mmps.tile([P, NF], FP32, tag="mm", name=f"mm{fc}")
                  for fc in range(FC)]
            for kt in range(0, KT, 2):
                for fc in range(FC):
                    nc.tensor.matmul(
                        mm[fc][:],
                        lhsT=sxt[:, kt:kt + 2, :],
                        rhs=sk[:, kt:kt + 2, fc * NF:(fc + 1) * NF],
                        start=(kt == 0), stop=(kt + 2 >= KT), perf_mode=pm)
            for fc in range(FC):
                sl = slice(fc * NF, (fc + 1) * NF)
                oc = outp.tile([P, NF], FP32, tag="out", name="oc")
                # y = (psum * bx[t]) * (0.25*(bk[f]+eps))
                nc.vector.scalar_tensor_tensor(
                    oc, mm[fc][:], bxall[:, j:j + 1], bkb[:, sl],
                    op0=ALU.mult, op1=ALU.mult)
                if has_bias:
                    nc.vector.tensor_tensor(oc, oc, biasb[:, sl], op=ALU.add)
                # y writes on the scalar ring: keeps them off the x-load
                # (sync) and weight (gpsimd) rings
                nc.scalar.dma_start(y_d[j * P:(j + 1) * P, sl], oc)

    if not nc.is_finalized():
        nc.finalize()
    return nc


def _pack_x(x2):
    """x2 [T, D] fp32 -> (xt [T, D] block-transposed, bx2 [P, TB])."""
    T, D = x2.shape
    KT, TB = D // P, T // P
    v = x2.reshape(TB, P, KT, P)                     # [j, t, kt, p]
    xt = np.ascontiguousarray(v.transpose(0, 3, 2, 1)).reshape(T, D)
    bx = (np.abs(x2).max(axis=1) + np.float32(F32_EPS)).astype(np.float32)
    bx2 = np.ascontiguousarray(bx.reshape(TB, P).T)  # [p, j]
    return xt, bx2


def _pack_k(ksh):
    """ksh [D, FS] fp32 -> (sk8 [P, KT, FS] fp8 signs, bkb [FS] fp32)."""
    D, FS = ksh.shape
    KT = D // P
    kv = ksh.reshape(KT, P, FS)                      # [kt, p, f]
    sk8 = np.ascontiguousarray(
        np.where(kv >= 0, np.float32(1.0), np.float32(-1.0))
        .astype(NP_FP8).transpose(1, 0, 2))          # [p, kt, f]
    bkb = ((np.abs(ksh).max(axis=0) + np.float32(F32_EPS))
           * np.float32(0.25)).astype(np.float32)
    return sk8, bkb


def _run(x2, k, b, has_bias, trace=False, **build_kwargs):
    """Host-pack inputs, compile once, run SPMD on all 8 cores."""
    T, D = x2.shape
    F = k.shape[1]
    FS = F // N_CORES
    xt, bx2 = _pack_x(x2)
    in_maps = []
    for c in range(N_CORES):
        sk8, bkb = _pack_k(np.ascontiguousarray(k[:, c * FS:(c + 1) * FS]))
        m = {"xt_in": xt, "sk_in": sk8, "bx_in": bx2, "bkb_in": bkb}
        if has_bias:
            m["b_in"] = np.ascontiguousarray(b[c * FS:(c + 1) * FS])
        in_maps.append(m)
    nc = build_nc(T, D, FS, has_bias=has_bias, **build_kwargs)
    res = bass_utils.run_bass_kernel_spmd(
        nc, in_maps, core_ids=list(range(N_CORES)), trace=trace)
    return res


def kernel(x, kernel, bias):
    x = np.ascontiguousarray(np.asarray(x, dtype=np.float32))
    k = np.ascontiguousarray(np.asarray(kernel, dtype=np.float32))
    b = np.ascontiguousarray(np.asarray(bias, dtype=np.float32))
    B, S, D = x.shape
    F = k.shape[1]
    T = B * S
    x2 = np.ascontiguousarray(x.reshape(T, D))
    has_bias = bool(np.any(b))
    res = _run(x2, k, b, has_bias)
    y = np.concatenate([res.results[c]["y_out"] for c in range(N_CORES)], axis=1)
    return np.ascontiguousarray(y.reshape(B, S, F)).astype(np.float32)


# revision 11
# speedup vs baseline: 1.0171x; 1.0171x over previous
"""BiDense (binary dense) kernel for Trainium2, column-parallel over 8 NeuronCores.

Math (mirrors the reference exactly):
    bk[f] = max_d |kernel[d, f]| + f32_eps          (per-output-feature bound)
    bx[t] = max_d |x[t, d]|      + f32_eps          (per-token bound)
    kq = sign*(kernel) * 0.5 * bk[f]                (sign* maps 0 -> +1)
    xq = sign*(x)      * 0.5 * bx[t]
    y[t, f] = sum_d xq kq + bias[f]
            = 0.25 * bx[t] * bk[f] * (Sx @ Sk)[t, f] + bias[f]

Sx/Sk are +-1 matrices, so the GEMM runs exactly in fp8 (products are +-1,
accumulation of <=4096 integers is exact in fp32 PSUM).

Layout strategy (v2): the host pre-packs data layouts so the device-side
program is a pure fp8 DoubleRow GEMM pipeline with no PE transposes and no
weight-bound reduction chain:
  - x is repacked (pure layout permutation) to x^T tiles [j, p, kt, t] so the
    matmul lhsT (d on partitions) can be produced by a single ACT sign pass
    per token block - the PE never transposes.
  - kernel signs are packed to fp8 [p, kt, f] on the host (weight
    quantization), shrinking the weight stream 4x and making the first
    matmul runnable within microseconds of kernel start.
  - the tiny per-token / per-feature bounds vectors (0.02% of the FLOPs)
    are computed host-side and DMA'd as constants, so PSUM evacuation is
    never blocked on a bounds reduction.

Sharding: column-parallel (tensor-parallel over features).  Each core gets
the full x and a 1/8 slice of kernel/bias along f; outputs concat along f.
"""

import numpy as np
import ml_dtypes
from contextlib import ExitStack

import concourse.bass as bass
import concourse.mybir as mybir
import concourse.tile as tile
from concourse import bacc, bass_utils

P = 128
N_CORES = 8
F32_EPS = float(np.finfo(np.float32).eps)
SIGN_BIAS = 1e-30  # sign(v + tiny): maps v==0 to +1, never flips a real value

FP32 = mybir.dt.float32
FP8 = mybir.dt.float8e4
NP_FP8 = ml_dtypes.float8_e4m3
ALU = mybir.AluOpType


def build_nc(T, D, F, has_bias=False, NF=512, XB=4, SKQ=16, PRE=4):
    """Per-core Bass program: pure fp8 DoubleRow GEMM pipeline.

    T: tokens (full T), D: contraction, F: features on this core.
    Inputs (host-packed):
      xt_in  [T, D] fp32   row j*P+p, col kt*P+t  =  x[j*P+t, kt*P+p]
      sk_in  [P, KT, F] fp8  sign(k[kt*P+p, f])
      bx_in  [P, TB] fp32   bx[j*P+p] at [p, j]
      bkb_in [F] fp32       0.25*(bk[f]+eps)
    """
    KT, TB, FC = D // P, T // P, F // NF
    assert T % P == 0 and D % P == 0 and F % NF == 0 and KT % 2 == 0
    assert KT % SKQ == 0
    PRE = min(PRE, TB)

    nc = bacc.Bacc(trn_type="TRN2")
    xt_d = nc.dram_tensor("xt_in", [T, D], FP32, kind="ExternalInput")
    sk_d = nc.dram_tensor("sk_in", [P, KT, F], FP8, kind="ExternalInput")
    bx_d = nc.dram_tensor("bx_in", [P, TB], FP32, kind="ExternalInput")
    bkb_d = nc.dram_tensor("bkb_in", [F], FP32, kind="ExternalInput")
    b_d = None
    if has_bias:
        b_d = nc.dram_tensor("b_in", [F], FP32, kind="ExternalInput")
    y_d = nc.dram_tensor("y_out", [T, F], FP32, kind="ExternalOutput")

    with ExitStack() as ctx:
        tc = ctx.enter_context(tile.TileContext(nc))
        const = ctx.enter_context(tc.tile_pool(name="const", bufs=1))
        skp = ctx.enter_context(tc.tile_pool(name="sk", bufs=1))
        mmps = ctx.enter_context(tc.tile_pool(name="mmps", bufs=8, space="PSUM"))
        xp = ctx.enter_context(tc.tile_pool(name="xp", bufs=XB))
        sxtp = ctx.enter_context(tc.tile_pool(name="sxtp", bufs=XB))
        outp = ctx.enter_context(tc.tile_pool(name="outp", bufs=8))

        # weight/const stream on the gpsimd DMA ring so the x loads on the
        # sync ring are never queued behind it; chunked so early kt tiles
        # land first and block 0 can accumulate while the tail is in flight
        bxall = const.tile([P, TB], FP32)
        nc.gpsimd.dma_start(bxall, bx_d[:, :])
        bkb = const.tile([P, F], FP32)
        src = bkb_d[:]
        bcast = bass.AP(tensor=src.tensor, offset=src.offset,
                        ap=[[0, P]] + [list(pair) for pair in src.ap])
        nc.gpsimd.dma_start(bkb, bcast)
        biasb = None
        if has_bias:
            biasb = const.tile([P, F], FP32, name="biasb")
            bsrc = b_d[:]
            bb = bass.AP(tensor=bsrc.tensor, offset=bsrc.offset,
                         ap=[[0, P]] + [list(pair) for pair in bsrc.ap])
            nc.gpsimd.dma_start(biasb, bb)
        sk = skp.tile([P, KT, F], FP8)
        kq = KT // SKQ
        sbias = const.tile([P, 1], FP32)   # tiny bias so sign(0+eps) = +1
        nc.vector.memset(sbias, SIGN_BIAS)

        pm = mybir.MatmulPerfMode.DoubleRow
        sxts = {}

        def emit_xload(j, chunks=1, eng=nc.sync):
            # chunks>1 pipelines DMA->sign at kt-group granularity so block
            # 0's first matmul isn't gated on the whole 2MB tile + sign
            xt = xp.tile([P, D], FP32, tag="xt", name="xt")
            sxt = sxtp.tile([P, KT, P], FP8, tag="sxt", name="sxt")
            cw = D // chunks
            kw = KT // chunks
            for c in range(chunks):
                eng.dma_start(xt[:, c * cw:(c + 1) * cw],
                              xt_d[j * P:(j + 1) * P, c * cw:(c + 1) * cw])
                nc.scalar.sign(
                    sxt[:, c * kw:(c + 1) * kw, :],
                    xt[:, c * cw:(c + 1) * cw].rearrange(
                        "p (k t) -> p k t", k=kw),
                    bias=sbias[:])
            sxts[j] = sxt

        # startup DMA priority, serialized on the gpsimd ring so eager
        # lookahead prefetches can't starve the weight stream: xt block 0
        # first (latency-critical for the first matmul), then the full
        # weight stream (block 0 consumes it in chunk order), then the
        # lookahead xt blocks.  Steady-state xt loads go on the sync ring.
        emit_xload(0, chunks=4, eng=nc.gpsimd)
        for q in range(SKQ):
            nc.gpsimd.dma_start(sk[:, q * kq:(q + 1) * kq, :],
                                sk_d[:, q * kq:(q + 1) * kq, :])
        for j in range(1, PRE):
            emit_xload(j, eng=nc.gpsimd)

        for j in range(TB):
            if j + PRE < TB:
                emit_xload(j + PRE)
            sxt = sxts.pop(j)
            mm = [mmps.tile([P, NF], FP32, tag="mm", name=f"mm{fc}")
                  for fc in range(FC)]
            for kt in range(0, KT, 2):
                for fc in range(FC):
                    nc.tensor.matmul(
                        mm[fc][:],
                        lhsT=sxt[:, kt:kt + 2, :],
                        rhs=sk[:, kt:kt + 2, fc * NF:(fc + 1) * NF],
                        start=(kt == 0), stop=(kt + 2 >= KT), perf_mode=pm)
            for fc in range(FC):
                sl = slice(fc * NF, (fc + 1) * NF)
                oc = outp.tile([P, NF], FP32, tag="out", name="oc")
                # y = (psum * bx[t]) * (0.25*(bk[f]+eps))
                nc.vector.scalar_tensor_tensor(
                    oc, mm[fc][:], bxall[:, j:j + 1], bkb[:, sl],
                    op0=ALU.mult, op1=ALU.mult)
                if has_bias:
                    nc.vector.tensor_tensor(oc, oc, biasb[:, sl], op=ALU.add)
                # y writes on the scalar ring: keeps them off the x-load
                # (sync) and weight (gpsimd) rings
                nc.scalar.dma_start(y_d[j * P:(j + 1) * P, sl], oc)

    if not nc.is_finalized():
        nc.finalize()
    return nc


def _pack_x(x2):
    """x2 [T, D] fp32 -> (xt [T, D] block-transposed, bx2 [P, TB])."""
    T, D = x2.shape
    KT, TB = D // P, T // P
    v = x2.reshape(TB, P, KT, P)                     # [j, t, kt, p]
    xt = np.ascontiguousarray(v.transpose(0, 3, 2, 1)).reshape(T, D)
    bx = (np.abs(x2).max(axis=1) + np.float32(F32_EPS)).astype(np.float32)
    bx2 = np.ascontiguousarray(bx.reshape(TB, P).T)  # [p, j]
    return xt, bx2


def _pack_k(ksh):
    """ksh [D, FS] fp32 -> (sk8 [P, KT, FS] fp8 signs, bkb [FS] fp32)."""
    D, FS = ksh.shape
    KT = D // P
    kv = ksh.reshape(KT, P, FS)                      # [kt, p, f]
    sk8 = np.ascontiguousarray(
        np.where(kv >= 0, np.float32(1.0), np.float32(-1.0))
        .astype(NP_FP8).transpose(1, 0, 2))          # [p, kt, f]
    bkb = ((np.abs(ksh).max(axis=0) + np.float32(F32_EPS))
           * np.float32(0.25)).astype(np.float32)
    return sk8, bkb


def _run(x2, k, b, has_bias, trace=False, **build_kwargs):
    """Host-pack inputs, compile once, run SPMD on all 8 cores."""
    T, D = x2.shape
    F = k.shape[1]
    FS = F // N_CORES
    xt, bx2 = _pack_x(x2)
    in_maps = []
    for c in range(N_CORES):
        sk8, bkb = _pack_k(np.ascontiguousarray(k[:, c * FS:(c + 1) * FS]))
        m = {"xt_in": xt, "sk_in": sk8, "bx_in": bx2, "bkb_in": bkb}
        if has_bias:
            m["b_in"] = np.ascontiguousarray(b[c * FS:(c + 1) * FS])
        in_maps.append(m)
    nc = build_nc(T, D, FS, has_bias=has_bias, **build_kwargs)
    res = bass_utils.run_bass_kernel_spmd(
        nc, in_maps, core_ids=list(range(N_CORES)), trace=trace)
    return res


def kernel(x, kernel, bias):
    x = np.ascontiguousarray(np.asarray(x, dtype=np.float32))
    k = np.ascontiguousarray(np.asarray(kernel, dtype=np.float32))
    b = np.ascontiguousarray(np.asarray(bias, dtype=np.float32))
    B, S, D = x.shape
    F = k.shape[1]
    T = B * S
    x2 = np.ascontiguousarray(x.reshape(T, D))
    has_bias = bool(np.any(b))
    res = _run(x2, k, b, has_bias)
    y = np.concatenate([res.results[c]["y_out"] for c in range(N_CORES)], axis=1)
    return np.ascontiguousarray(y.reshape(B, S, F)).astype(np.float32)
